# revision 32
# baseline (speedup 1.0000x reference)
"""Multi-head attention (B=4, S=2048, D=1024, H=16) on 8 trn2 NeuronCores.

Sharding: data-parallel over batch (4) x tensor-parallel over heads (2 groups
of 8 heads).  Core c handles batch b=c//2, head group g=c%2: it gets
Wq/Wk/Wv[:, g*512:(g+1)*512] and Wo[g*512:(g+1)*512, :] and produces a partial
output [S, D]; the host sums the two partials of each batch (the row-split of
Wo makes the full output an exact sum of the two group partials).

Per-core kernel (matmuls in float32r; every matmul operand is materialized as
rounded float32r to satisfy the BIR verifier):
  1. PE-transpose x -> xT [D, S] (fp32)     (contraction needs D on partitions)
  2. V  = x @ wv            [S, 512] (+ per-head ones column for row sums)
     QT = (x @ wq)^T        [512, S]
     KT = (x @ wk)^T        [512, S]
  3. per head: scoresT[k,q] = KT_h-slices.T @ QT_h  (K=64)
     exp fused with mask bias + 1/sqrt(dk) scale on ACT
     PV in transposed form: outT[65, q] += V_h(+ones).T @ expT, row 64
     accumulates the softmax denominators; normalize with DVE reciprocal +
     gpsimd partition-broadcast.
  4. y = outT.T @ wo        [S, D]
"""

import os
import sys

import numpy as np

_TRN_REPO = "/opt/trn_rl_repo"
if _TRN_REPO not in sys.path:
    sys.path.insert(0, _TRN_REPO)

from contextlib import ExitStack

import concourse.bass as bass
import concourse.mybir as mybir
import concourse.tile as tile
from concourse.masks import make_identity
from concourse import library_config
from concourse.bass_utils import run_bass_kernel_spmd

S, D, H, DK = 2048, 1024, 16, 64
NCORES = 8
HG = 2                # head-parallel groups
B = 4                 # batches
H8 = H // HG          # heads per core
C = H8 * DK           # 512: per-core projection width
P = 128
KT = D // P           # 8  k-tiles over D
ST = S // P           # 16 tiles over S
CT = C // P           # 4  tiles over C
VW = DK + 1           # 65: v columns + ones column
QC = 512              # q-chunk in attention phase (head-pair scheme)
NQC = S // QC

f32 = mybir.dt.float32
f32r = mybir.dt.float32r
i32 = mybir.dt.int32
FT = mybir.ActivationFunctionType
ALU = mybir.AluOpType


def build_nc(split_waits=True):
    nc = bass.Bass()
    x_d = nc.declare_dram_parameter("x", [S, D], f32, isOutput=False)
    wq_d = nc.declare_dram_parameter("wq", [D, C], f32r, isOutput=False)
    wk_d = nc.declare_dram_parameter("wk", [D, C], f32r, isOutput=False)
    wv_d = nc.declare_dram_parameter("wv", [D, C], f32r, isOutput=False)
    wo_d = nc.declare_dram_parameter("wo", [C, D], f32r, isOutput=False)
    mask_d = nc.declare_dram_parameter("maskt", [P, ST], i32, isOutput=False)
    y_d = nc.declare_dram_parameter("y", [S, D], f32, isOutput=True)

    with tile.TileContext(nc) as tc, ExitStack() as ctx:
        perm = ctx.enter_context(tc.tile_pool(name="perm", bufs=1))
        ident = perm.tile([P, P], f32)
        make_identity(nc, ident)

        # mask bias: (m - 1) * 1e9 per key, keys on partitions, one col per k-tile
        mask_i = perm.tile([P, ST], i32)
        nc.sync.dma_start(mask_i, mask_d[:, :])
        mask_b = perm.tile([P, ST], f32)
        nc.vector.tensor_copy(mask_b, mask_i)
        nc.vector.tensor_scalar(mask_b, mask_b, -1.0, 1.0e9, ALU.add, ALU.mult)

        QT = perm.tile([P, CT, S], f32r)
        KTl = perm.tile([P, CT, S], f32r)
        V = perm.tile([P, ST, H8 * VW], f32r)
        V4 = V.rearrange("p st (h w) -> p st h w", w=VW)
        # ones columns (col 64 of each head block) via rounding copy from an
        # f32 scratch tile (f32r memset is invalid ISA)
        ones_sc = perm.tile([P, 1], f32)
        nc.vector.memset(ones_sc[:, :], 1.0)
        V3 = V.rearrange("p st (h w) -> p (st h) w", w=VW)
        nc.vector.tensor_copy(
            V3[:, :, DK : DK + 1], ones_sc[:, :, None].to_broadcast((P, P, 1))
        )

        with tc.tile_pool(name="xTp", bufs=1) as xTp:
            xT = xTp.tile([P, KT, S], f32r)

            # ---- phase 1: x -> xT via PE transpose (fp32), rounding copy out
            with (
                tc.tile_pool(name="xload", bufs=4) as xp,
                tc.tile_pool(name="tpps", bufs=4, space="PSUM") as tpp,
            ):
                for st in range(ST):
                    xt = xp.tile([P, D], f32, tag="x")
                    nc.sync.dma_start(xt, x_d[st * P : (st + 1) * P, :])
                    for kt in range(KT):
                        ps = tpp.tile([P, P], f32, tag="tp")
                        nc.tensor.transpose(
                            ps, xt[:, kt * P : (kt + 1) * P], ident
                        )
                        nc.any.tensor_copy(xT[:, kt, st * P : (st + 1) * P], ps)

            with (
                tc.tile_pool(name="pps", bufs=4, space="PSUM") as pp,
                tc.tile_pool(name="wts", bufs=2) as wts,
            ):
                # ---- phase 2a: V = x @ wv  (weight tiles double-buffered so
                # the next weight's DMA overlaps the current projections)
                wv_sb = wts.tile([P, KT, C], f32r, tag="w")
                nc.sync.dma_start(wv_sb, wv_d.rearrange("(kt p) c -> p kt c", p=P))
                for st in range(ST):
                    ps = pp.tile([P, C], f32, tag="mm")
                    for kt in range(KT):
                        nc.tensor.matmul(
                            ps,
                            xT[:, kt, st * P : (st + 1) * P],
                            wv_sb[:, kt, :],
                            start=(kt == 0),
                            stop=(kt == KT - 1),
                        )
                    nc.any.tensor_copy(
                        V4[:, st, :, 0:DK],
                        ps.rearrange("p (h w) -> p h w", w=DK),
                    )

                # ---- phase 2b: QT, KT
                for wd, dst in ((wq_d, QT), (wk_d, KTl)):
                    w_sb = wts.tile([P, KT, C], f32r, tag="w")
                    nc.sync.dma_start(
                        w_sb, wd.rearrange("(kt p) c -> p kt c", p=P)
                    )
                    for ct in range(CT):
                        for sch in range(S // 512):
                            ps = pp.tile([P, C], f32, tag="mm")
                            for kt in range(KT):
                                nc.tensor.matmul(
                                    ps,
                                    w_sb[:, kt, ct * P : (ct + 1) * P],
                                    xT[:, kt, sch * 512 : (sch + 1) * 512],
                                    start=(kt == 0),
                                    stop=(kt == KT - 1),
                                )
                            nc.any.tensor_copy(
                                dst[:, ct, sch * 512 : (sch + 1) * 512], ps
                            )

        # ---- phase 3: attention, one head PAIR at a time.
        # heads 2*pt (partitions 0:64) and 2*pt+1 (partitions 64:128) run their
        # scoresT matmuls CONCURRENTLY on row groups (0,0)/(64,0); one ACT exp
        # covers both heads' stripes; PV accumulates each head's outT[65, 512]
        # in its own PSUM bank (8 banks exactly, all double-buffered).
        otsb = ctx.enter_context(tc.tile_pool(name="otsb", bufs=1))
        outT = otsb.tile([P, CT, S], f32r)
        # 32 (head, q-chunk) row-sum vectors packed at start partitions
        # {0,32,64,96} x 8 column blocks (engine SBUF APs must start at k*32)
        rowsums = otsb.tile([P, H8 * NQC // 4, QC], f32)
        nc.vector.memset(rowsums[:, :, :], 1.0)
        wo_sb = otsb.tile([P, CT, D], f32r)
        nc.sync.dma_start(wo_sb, wo_d.rearrange("(pt p) e -> p pt e", p=P))
        # q-chunk OUTER loop: after all 4 pairs finish a q-chunk, that chunk
        # is normalized and its y = outT.T @ wo slice computed + stored while
        # the next q-chunk's (ACT-bound) attention runs -- phase 4 is fully
        # absorbed into phase 3.
        with (
            tc.tile_pool(name="scps", bufs=2, space="PSUM") as scp,
            tc.tile_pool(name="otps", bufs=2, space="PSUM") as otp,
            tc.tile_pool(name="rsyps", bufs=2, space="PSUM") as rsy,
            tc.tile_pool(name="expool", bufs=4) as exp_pool,
            tc.tile_pool(name="bcp", bufs=4) as bcp,
            tc.tile_pool(name="ypool", bufs=4) as ypl,
            tc.tile_pool(name="rsd", bufs=2, space="DRAM") as rsd,
        ):
            for qc in range(NQC):
                qs = slice(qc * QC, (qc + 1) * QC)
                for pt in range(CT):
                    h0, h1 = 2 * pt, 2 * pt + 1
                    ot0 = otp.tile([VW, QC], f32, tag="ot")
                    ot1 = otp.tile([VW, QC], f32, tag="ot")
                    for kt in range(ST):
                        sc_ps = scp.tile([P, 2, QC], f32, tag="sc")
                        nc.tensor.matmul(
                            sc_ps[:, 0, :],
                            KTl[0:DK, pt, kt * P : (kt + 1) * P],
                            QT[0:DK, pt, qs],
                            start=True,
                            stop=True,
                            tile_position=(0, 0),
                        )
                        nc.tensor.matmul(
                            sc_ps[:, 1, :],
                            KTl[DK:P, pt, kt * P : (kt + 1) * P],
                            QT[DK:P, pt, qs],
                            start=True,
                            stop=True,
                            tile_position=(64, 0),
                        )
                        ex = exp_pool.tile([P, 2, QC], f32r, tag="ex")
                        nc.scalar.activation(
                            ex,
                            sc_ps,
                            FT.Exp,
                            bias=mask_b[:, kt : kt + 1],
                            scale=0.125,
                        )
                        nc.tensor.matmul(
                            ot0,
                            V4[:, kt, h0, :],
                            ex[:, 0, :],
                            start=(kt == 0),
                            stop=(kt == ST - 1),
                        )
                        nc.tensor.matmul(
                            ot1,
                            V4[:, kt, h1, :],
                            ex[:, 1, :],
                            start=(kt == 0),
                            stop=(kt == ST - 1),
                        )
                    # rowsum vector (h, qc) at row (h%4)*32, block qc*2 + h//4
                    for half, ot in ((0, ot0), (1, ot1)):
                        h = 2 * pt + half
                        nc.vector.tensor_copy(
                            rowsums[
                                (h % 4) * 32 : (h % 4) * 32 + 1,
                                2 * qc + h // 4,
                                :,
                            ],
                            ot[DK : DK + 1, :],
                        )
                        nc.vector.tensor_copy(
                            outT[half * DK : (half + 1) * DK, pt, qs],
                            ot[0:DK, :],
                        )

                # normalize this q-chunk across all 8 heads (recip on ACT via
                # exp(-ln(x)); unused lanes hold memset 1.0 -> 1.0)
                rsp = rowsums[:, 2 * qc : 2 * qc + 2, :]
                nc.scalar.activation(rsp, rsp, FT.Ln)
                nc.scalar.activation(rsp, rsp, FT.Exp, scale=-1.0)
                rs_dram = rsd.tile([H8, QC], f32, tag="rsd")
                for h in range(H8):
                    nc.sync.dma_start(
                        rs_dram[h : h + 1, :],
                        rowsums[
                            (h % 4) * 32 : (h % 4) * 32 + 1, 2 * qc + h // 4, :
                        ],
                    )
                for pt in range(CT):
                    bc = bcp.tile([P, QC], f32, tag="bc")
                    for half in range(2):
                        nc.sync.dma_start(
                            bc[half * DK : (half + 1) * DK, :],
                            rs_dram[
                                2 * pt + half : 2 * pt + half + 1, :
                            ].to_broadcast((DK, QC)),
                        )
                    nc.vector.tensor_mul(
                        outT[:, pt, qs], outT[:, pt, qs], bc
                    )

                # y for this q-chunk's 4 s-tiles, overlapping next chunk
                for sti in range(QC // P):
                    st = qc * (QC // P) + sti
                    y_sb = ypl.tile([P, D], f32, tag="y")
                    for ec in range(D // 512):
                        ps = rsy.tile([P, QC], f32, tag="rsy")
                        for pt in range(CT):
                            nc.tensor.matmul(
                                ps,
                                outT[:, pt, st * P : (st + 1) * P],
                                wo_sb[:, pt, ec * 512 : (ec + 1) * 512],
                                start=(pt == 0),
                                stop=(pt == CT - 1),
                            )
                        nc.any.tensor_copy(
                            y_sb[:, ec * 512 : (ec + 1) * 512], ps
                        )
                        nc.sync.dma_start(
                            y_d[st * P : (st + 1) * P, ec * 512 : (ec + 1) * 512],
                            y_sb[:, ec * 512 : (ec + 1) * 512],
                        )

    if split_waits:
        _split_matmul_waits(nc)
    return nc


def _split_matmul_waits(nc):
    """fp32/f32r matmuls (and DMA descriptors) lower to structs that hold
    only ONE sync wait; move extra waits onto a nop on the same engine."""
    import bass_rust

    n = 0
    for f in nc.m.functions:
        for blk in f.blocks:
            out = []
            for inst in blk.instructions:
                si = getattr(inst, "sync_info", None)
                if si is not None and len(si.on_wait) > 1:
                    waits = list(si.on_wait)
                    for w in waits[:-1]:
                        nop = bass_rust.InstNoOp(
                            name=f"I-mmw{n}", ins=[], outs=[], engine=inst.engine
                        )
                        n += 1
                        nop.sync_info = bass_rust.SyncInfo(
                            on_wait=[w], on_update=[]
                        )
                        out.append(nop)
                    inst.sync_info = bass_rust.SyncInfo(
                        on_wait=waits[-1:], on_update=list(si.on_update)
                    )
                out.append(inst)
            blk.instructions = out
    return nc


_NC_CACHE = None


def get_nc():
    global _NC_CACHE
    if _NC_CACHE is None:
        _NC_CACHE = build_nc()
    return _NC_CACHE


def make_in_maps(inputs):
    inp = np.asarray(inputs["inputs"], dtype=np.float32)
    mask = np.asarray(inputs["mask"], dtype=np.int32)
    Wq = np.asarray(inputs["Wq"], dtype=np.float32)
    Wk = np.asarray(inputs["Wk"], dtype=np.float32)
    Wv = np.asarray(inputs["Wv"], dtype=np.float32)
    Wo = np.asarray(inputs["Wo"], dtype=np.float32)

    in_maps = []
    for c in range(NCORES):
        b, g = c // HG, c % HG
        cs = slice(g * C, (g + 1) * C)
        in_maps.append(
            {
                "x": np.ascontiguousarray(inp[b]),
                "wq": np.ascontiguousarray(Wq[:, cs]),
                "wk": np.ascontiguousarray(Wk[:, cs]),
                "wv": np.ascontiguousarray(Wv[:, cs]),
                "wo": np.ascontiguousarray(Wo[cs, :]),
                "maskt": np.ascontiguousarray(mask[b].reshape(ST, P).T),
            }
        )
    return in_maps


def gather(results):
    out = np.empty((B, S, D), np.float32)
    for b in range(B):
        out[b] = results[HG * b]["y"] + results[HG * b + 1]["y"]
    return out


def run(inputs, **kwargs):
    """Run on hardware; returns (output, BassKernelResults)."""
    res = run_bass_kernel_spmd(
        get_nc(), make_in_maps(inputs), list(range(NCORES)), **kwargs
    )
    return gather(res.results), res


def kernel(**inputs) -> np.ndarray:
    out, _ = run(inputs)
    return out


# revision 33
# speedup vs baseline: 1.0220x; 1.0220x over previous
"""Multi-head attention (B=4, S=2048, D=1024, H=16) on 8 trn2 NeuronCores.

Sharding: data-parallel over batch (4) x tensor-parallel over heads (2 groups
of 8 heads).  Core c handles batch b=c//2, head group g=c%2: it gets
Wq/Wk/Wv[:, g*512:(g+1)*512] and Wo[g*512:(g+1)*512, :] and produces a partial
output [S, D]; the host sums the two partials of each batch (the row-split of
Wo makes the full output an exact sum of the two group partials).

Per-core kernel (matmuls in float32r = 1 cyc/row; every matmul operand is
materialized as rounded float32r to satisfy the BIR verifier):
  1. PE-transpose x -> xT [D, S] (fp32)     (contraction needs D on partitions)
  2. V  = x @ wv            [S, 512] (+ per-head ones column for row sums)
     QT = (x @ wq)^T        [512, S]    (weight DMAs double-buffered)
     KT = (x @ wk)^T        [512, S]
  3. attention with q-chunk outer loop, head PAIRS inner: the two heads of a
     partition tile run their K=64 scoresT matmuls CONCURRENTLY on PE row
     groups (0,0)/(64,0) via tile_position (2x); one ACT instr does
     exp(s/8 + maskbias) for both heads; PV in transposed form
     outT[65, q] += V_h(+ones).T @ expT accumulates values + softmax
     denominators; per q-chunk: recip via exp(-ln(x)) on ACT, DRAM-bounce
     partition-broadcast, in-place normalize, then that q-chunk's
     y = outT.T @ wo runs inside the ACT-bound attention phase.
  4. (absorbed into 3)
The kernel is ~ACT-bound: 33.5M softmax exps/core at 1 elem/lane/cycle
@1.2GHz is a ~240us floor; PE work (~275us busy) overlaps it.
"""

import os
import sys

import numpy as np

_TRN_REPO = "/opt/trn_rl_repo"
if _TRN_REPO not in sys.path:
    sys.path.insert(0, _TRN_REPO)

from contextlib import ExitStack

import concourse.bass as bass
import concourse.mybir as mybir
import concourse.tile as tile
from concourse.masks import make_identity
from concourse import library_config
from concourse.bass_utils import run_bass_kernel_spmd

# If BASS_TRACE is set in the environment, run_bass_kernel_spmd imports
# antenv.axon_hooks, which this container image lacks -- pre-install a stub
# so kernel() degrades to an untraced run instead of crashing.  test.py
# overwrites the stub with a real ctypes-backed hook for profiling.
if "antenv.axon_hooks" not in sys.modules:
    try:
        import antenv.axon_hooks  # noqa: F401
    except Exception:
        import types as _types

        _hookmod = _types.ModuleType("antenv.axon_hooks")
        _hookstore = {}
        _hookmod.set_axon_ntff_profile_hook = lambda h: _hookstore.__setitem__(
            "h", h
        )
        _hookmod.get_axon_ntff_profile_hook = lambda: _hookstore.get("h")
        sys.modules["antenv.axon_hooks"] = _hookmod
        try:
            import antenv

            antenv.axon_hooks = _hookmod
        except Exception:
            pass

S, D, H, DK = 2048, 1024, 16, 64
NCORES = 8
HG = 2                # head-parallel groups
B = 4                 # batches
H8 = H // HG          # heads per core
C = H8 * DK           # 512: per-core projection width
P = 128
KT = D // P           # 8  k-tiles over D
ST = S // P           # 16 tiles over S
CT = C // P           # 4  tiles over C
VW = DK + 1           # 65: v columns + ones column
QC = 512              # q-chunk in attention phase (head-pair scheme)
NQC = S // QC

f32 = mybir.dt.float32
f32r = mybir.dt.float32r
i32 = mybir.dt.int32
FT = mybir.ActivationFunctionType
ALU = mybir.AluOpType


def build_nc(split_waits=True):
    nc = bass.Bass()
    x_d = nc.declare_dram_parameter("x", [S, D], f32, isOutput=False)
    wq_d = nc.declare_dram_parameter("wq", [D, C], f32r, isOutput=False)
    wk_d = nc.declare_dram_parameter("wk", [D, C], f32r, isOutput=False)
    wv_d = nc.declare_dram_parameter("wv", [D, C], f32r, isOutput=False)
    wo_d = nc.declare_dram_parameter("wo", [C, D], f32r, isOutput=False)
    mask_d = nc.declare_dram_parameter("maskt", [P, ST], i32, isOutput=False)
    y_d = nc.declare_dram_parameter("y", [S, D], f32, isOutput=True)

    with tile.TileContext(nc) as tc, ExitStack() as ctx:
        perm = ctx.enter_context(tc.tile_pool(name="perm", bufs=1))
        ident = perm.tile([P, P], f32)
        make_identity(nc, ident)

        # mask bias: (m - 1) * 1e9 per key, keys on partitions, one col per k-tile
        mask_i = perm.tile([P, ST], i32)
        nc.sync.dma_start(mask_i, mask_d[:, :])
        mask_b = perm.tile([P, ST], f32)
        nc.vector.tensor_copy(mask_b, mask_i)
        nc.vector.tensor_scalar(mask_b, mask_b, -1.0, 1.0e9, ALU.add, ALU.mult)

        QT = perm.tile([P, CT, S], f32r)
        KTl = perm.tile([P, CT, S], f32r)
        V = perm.tile([P, ST, H8 * VW], f32r)
        V4 = V.rearrange("p st (h w) -> p st h w", w=VW)
        # ones columns (col 64 of each head block) via rounding copy from an
        # f32 scratch tile (f32r memset is invalid ISA)
        ones_sc = perm.tile([P, 1], f32)
        nc.vector.memset(ones_sc[:, :], 1.0)
        V3 = V.rearrange("p st (h w) -> p (st h) w", w=VW)
        nc.vector.tensor_copy(
            V3[:, :, DK : DK + 1], ones_sc[:, :, None].to_broadcast((P, P, 1))
        )

        with tc.tile_pool(name="xTp", bufs=1) as xTp:
            xT = xTp.tile([P, KT, S], f32r)

            # ---- phase 1: x -> xT via PE transpose (fp32), rounding copy out
            with (
                tc.tile_pool(name="xload", bufs=4) as xp,
                tc.tile_pool(name="tpps", bufs=4, space="PSUM") as tpp,
            ):
                for st in range(ST):
                    xt = xp.tile([P, D], f32, tag="x")
                    nc.sync.dma_start(xt, x_d[st * P : (st + 1) * P, :])
                    for kt in range(KT):
                        ps = tpp.tile([P, P], f32, tag="tp")
                        nc.tensor.transpose(
                            ps, xt[:, kt * P : (kt + 1) * P], ident
                        )
                        nc.any.tensor_copy(xT[:, kt, st * P : (st + 1) * P], ps)

            with (
                tc.tile_pool(name="pps", bufs=4, space="PSUM") as pp,
                tc.tile_pool(name="wts", bufs=2) as wts,
            ):
                # ---- phase 2a: V = x @ wv  (weight tiles double-buffered so
                # the next weight's DMA overlaps the current projections)
                wv_sb = wts.tile([P, KT, C], f32r, tag="w")
                nc.sync.dma_start(wv_sb, wv_d.rearrange("(kt p) c -> p kt c", p=P))
                for st in range(ST):
                    ps = pp.tile([P, C], f32, tag="mm")
                    for kt in range(KT):
                        nc.tensor.matmul(
                            ps,
                            xT[:, kt, st * P : (st + 1) * P],
                            wv_sb[:, kt, :],
                            start=(kt == 0),
                            stop=(kt == KT - 1),
                        )
                    nc.any.tensor_copy(
                        V4[:, st, :, 0:DK],
                        ps.rearrange("p (h w) -> p h w", w=DK),
                    )

                # ---- phase 2b: QT, KT
                for wd, dst in ((wq_d, QT), (wk_d, KTl)):
                    w_sb = wts.tile([P, KT, C], f32r, tag="w")
                    nc.sync.dma_start(
                        w_sb, wd.rearrange("(kt p) c -> p kt c", p=P)
                    )
                    for ct in range(CT):
                        for sch in range(S // 512):
                            ps = pp.tile([P, C], f32, tag="mm")
                            for kt in range(KT):
                                nc.tensor.matmul(
                                    ps,
                                    w_sb[:, kt, ct * P : (ct + 1) * P],
                                    xT[:, kt, sch * 512 : (sch + 1) * 512],
                                    start=(kt == 0),
                                    stop=(kt == KT - 1),
                                )
                            nc.any.tensor_copy(
                                dst[:, ct, sch * 512 : (sch + 1) * 512], ps
                            )

        # ---- phase 3: attention, one head PAIR at a time.
        # heads 2*pt (partitions 0:64) and 2*pt+1 (partitions 64:128) run their
        # scoresT matmuls CONCURRENTLY on row groups (0,0)/(64,0); one ACT exp
        # covers both heads' stripes; PV accumulates each head's outT[65, 512]
        # in its own PSUM bank (8 banks exactly, all double-buffered).
        otsb = ctx.enter_context(tc.tile_pool(name="otsb", bufs=1))
        outT = otsb.tile([P, CT, S], f32r)
        # 32 (head, q-chunk) row-sum vectors packed at start partitions
        # {0,32,64,96} x 8 column blocks (engine SBUF APs must start at k*32)
        rowsums = otsb.tile([P, H8 * NQC // 4, QC], f32)
        nc.vector.memset(rowsums[:, :, :], 1.0)
        wo_sb = otsb.tile([P, CT, D], f32r)
        nc.sync.dma_start(wo_sb, wo_d.rearrange("(pt p) e -> p pt e", p=P))
        # q-chunk OUTER loop: after all 4 pairs finish a q-chunk, that chunk
        # is normalized and its y = outT.T @ wo slice computed + stored while
        # the next q-chunk's (ACT-bound) attention runs -- phase 4 is fully
        # absorbed into phase 3.
        with (
            tc.tile_pool(name="scps", bufs=2, space="PSUM") as scp,
            tc.tile_pool(name="otps", bufs=2, space="PSUM") as otp,
            tc.tile_pool(name="rsyps", bufs=2, space="PSUM") as rsy,
            tc.tile_pool(name="expool", bufs=4) as exp_pool,
            tc.tile_pool(name="bcp", bufs=4) as bcp,
            tc.tile_pool(name="ypool", bufs=4) as ypl,
            tc.tile_pool(name="rsd", bufs=2, space="DRAM") as rsd,
        ):
            for qc in range(NQC):
                qs = slice(qc * QC, (qc + 1) * QC)
                for pt in range(CT):
                    h0, h1 = 2 * pt, 2 * pt + 1
                    ot0 = otp.tile([VW, QC], f32, tag="ot")
                    ot1 = otp.tile([VW, QC], f32, tag="ot")
                    for kt in range(ST):
                        sc_ps = scp.tile([P, 2, QC], f32, tag="sc")
                        nc.tensor.matmul(
                            sc_ps[:, 0, :],
                            KTl[0:DK, pt, kt * P : (kt + 1) * P],
                            QT[0:DK, pt, qs],
                            start=True,
                            stop=True,
                            tile_position=(0, 0),
                        )
                        nc.tensor.matmul(
                            sc_ps[:, 1, :],
                            KTl[DK:P, pt, kt * P : (kt + 1) * P],
                            QT[DK:P, pt, qs],
                            start=True,
                            stop=True,
                            tile_position=(64, 0),
                        )
                        ex = exp_pool.tile([P, 2, QC], f32r, tag="ex")
                        nc.scalar.activation(
                            ex,
                            sc_ps,
                            FT.Exp,
                            bias=mask_b[:, kt : kt + 1],
                            scale=0.125,
                        )
                        nc.tensor.matmul(
                            ot0,
                            V4[:, kt, h0, :],
                            ex[:, 0, :],
                            start=(kt == 0),
                            stop=(kt == ST - 1),
                        )
                        nc.tensor.matmul(
                            ot1,
                            V4[:, kt, h1, :],
                            ex[:, 1, :],
                            start=(kt == 0),
                            stop=(kt == ST - 1),
                        )
                    # rowsum vector (h, qc) at row (h%4)*32, block qc*2 + h//4
                    for half, ot in ((0, ot0), (1, ot1)):
                        h = 2 * pt + half
                        nc.vector.tensor_copy(
                            rowsums[
                                (h % 4) * 32 : (h % 4) * 32 + 1,
                                2 * qc + h // 4,
                                :,
                            ],
                            ot[DK : DK + 1, :],
                        )
                        nc.vector.tensor_copy(
                            outT[half * DK : (half + 1) * DK, pt, qs],
                            ot[0:DK, :],
                        )

                # normalize this q-chunk across all 8 heads (recip on ACT via
                # exp(-ln(x)); unused lanes hold memset 1.0 -> 1.0)
                rsp = rowsums[:, 2 * qc : 2 * qc + 2, :]
                nc.scalar.activation(rsp, rsp, FT.Ln)
                nc.scalar.activation(rsp, rsp, FT.Exp, scale=-1.0)
                rs_dram = rsd.tile([H8, QC], f32, tag="rsd")
                for h in range(H8):
                    nc.sync.dma_start(
                        rs_dram[h : h + 1, :],
                        rowsums[
                            (h % 4) * 32 : (h % 4) * 32 + 1, 2 * qc + h // 4, :
                        ],
                    )
                for pt in range(CT):
                    bc = bcp.tile([P, QC], f32, tag="bc")
                    for half in range(2):
                        nc.sync.dma_start(
                            bc[half * DK : (half + 1) * DK, :],
                            rs_dram[
                                2 * pt + half : 2 * pt + half + 1, :
                            ].to_broadcast((DK, QC)),
                        )
                    nc.vector.tensor_mul(
                        outT[:, pt, qs], outT[:, pt, qs], bc
                    )

                # y for this q-chunk's 4 s-tiles, overlapping next chunk
                for sti in range(QC // P):
                    st = qc * (QC // P) + sti
                    y_sb = ypl.tile([P, D], f32, tag="y")
                    for ec in range(D // 512):
                        ps = rsy.tile([P, QC], f32, tag="rsy")
                        for pt in range(CT):
                            nc.tensor.matmul(
                                ps,
                                outT[:, pt, st * P : (st + 1) * P],
                                wo_sb[:, pt, ec * 512 : (ec + 1) * 512],
                                start=(pt == 0),
                                stop=(pt == CT - 1),
                            )
                        nc.any.tensor_copy(
                            y_sb[:, ec * 512 : (ec + 1) * 512], ps
                        )
                        nc.sync.dma_start(
                            y_d[st * P : (st + 1) * P, ec * 512 : (ec + 1) * 512],
                            y_sb[:, ec * 512 : (ec + 1) * 512],
                        )

    if split_waits:
        _split_matmul_waits(nc)
    return nc


def _split_matmul_waits(nc):
    """fp32/f32r matmuls (and DMA descriptors) lower to structs that hold
    only ONE sync wait; move extra waits onto a nop on the same engine."""
    import bass_rust

    n = 0
    for f in nc.m.functions:
        for blk in f.blocks:
            out = []
            for inst in blk.instructions:
                si = getattr(inst, "sync_info", None)
                if si is not None and len(si.on_wait) > 1:
                    waits = list(si.on_wait)
                    for w in waits[:-1]:
                        nop = bass_rust.InstNoOp(
                            name=f"I-mmw{n}", ins=[], outs=[], engine=inst.engine
                        )
                        n += 1
                        nop.sync_info = bass_rust.SyncInfo(
                            on_wait=[w], on_update=[]
                        )
                        out.append(nop)
                    inst.sync_info = bass_rust.SyncInfo(
                        on_wait=waits[-1:], on_update=list(si.on_update)
                    )
                out.append(inst)
            blk.instructions = out
    return nc


_NC_CACHE = None


def get_nc():
    global _NC_CACHE
    if _NC_CACHE is None:
        _NC_CACHE = build_nc()
    return _NC_CACHE


def make_in_maps(inputs):
    inp = np.asarray(inputs["inputs"], dtype=np.float32)
    mask = np.asarray(inputs["mask"], dtype=np.int32)
    Wq = np.asarray(inputs["Wq"], dtype=np.float32)
    Wk = np.asarray(inputs["Wk"], dtype=np.float32)
    Wv = np.asarray(inputs["Wv"], dtype=np.float32)
    Wo = np.asarray(inputs["Wo"], dtype=np.float32)

    in_maps = []
    for c in range(NCORES):
        b, g = c // HG, c % HG
        cs = slice(g * C, (g + 1) * C)
        in_maps.append(
            {
                "x": np.ascontiguousarray(inp[b]),
                "wq": np.ascontiguousarray(Wq[:, cs]),
                "wk": np.ascontiguousarray(Wk[:, cs]),
                "wv": np.ascontiguousarray(Wv[:, cs]),
                "wo": np.ascontiguousarray(Wo[cs, :]),
                "maskt": np.ascontiguousarray(mask[b].reshape(ST, P).T),
            }
        )
    return in_maps


def gather(results):
    out = np.empty((B, S, D), np.float32)
    for b in range(B):
        out[b] = results[HG * b]["y"] + results[HG * b + 1]["y"]
    return out


def run(inputs, **kwargs):
    """Run on hardware; returns (output, BassKernelResults)."""
    res = run_bass_kernel_spmd(
        get_nc(), make_in_maps(inputs), list(range(NCORES)), **kwargs
    )
    return gather(res.results), res


def kernel(**inputs) -> np.ndarray:
    out, _ = run(inputs)
    return out
